# revision 3
# baseline (speedup 1.0000x reference)
"""Contrastive loss (SupCon-style) on 8 Trainium2 NeuronCores.

Reference (N=8192, D=1024, T=0.1):
    sim = emb @ emb.T / T;  e = exp(sim)
    all_sum_i = sum_j e_ij - e_ii
    pos_sum_i = sum_j e_ij * lab_j - e_ii * lab_i
    loss = mean_{i: lab_i=1} [ log(all_sum_i + eps) - log(pos_sum_i) ]
    (0.0 if n_ref < 2)

Strategy: rows are pre-sorted by label on the host (the loss is invariant
under a joint row+column permutation of the similarity matrix), then split
across 8 cores (1024 rows each). Each core gets a column-ROTATED copy of the
sorted emb^T so its diagonal block lands at fixed local columns [0, 1024) --
identical SPMD instruction stream, row sums invariant under the rotation.

Because columns are label-sorted-then-rotated, every 1024-column slice is
label-constant except at one in-slice offset b = n0 % 1024 (identical for
all cores). Each exp activation is split at b, so every per-slice row-sum
emitted by the ScalarE accumulator is label-pure; pos_sum is then just a
per-core weighted reduction of the accumulator stats. This removes the
whole VectorE pos-sum pass over the [N/8, N] exp matrix that the v1 kernel
ran (it was ~40% of device time on HW).

Per core: sim tile [128 i x 512 j] = 4 accumulating fp8 DoubleRow matmuls;
diagonal positions get -BIG added pre-exp (exact self-exclusion); ScalarE
exp(scale*x) with accum_out emits the per-slice-piece row sums; final
log/mask/reduce on device down to one partial scalar per core; host sums 8
partials / n_ref.
"""

import numpy as np

import concourse.bass as bass
import concourse.tile as tile
import concourse.mybir as mybir
from concourse import bacc
from concourse.bass_utils import run_bass_kernel_spmd

N, D = 8192, 1024
NCORES = 8
ROWS = N // NCORES  # 1024 rows per core
P = 128             # partitions
JS = 512            # j-slice width (one PSUM bank of fp32)
NJ2 = N // (2 * JS)  # 8 j slices of 1024
ND = D // P         # 8 contraction chunks
IC = ROWS // P      # 8 row chunks per core
SCALE = 10.0        # 1 / TEMPERATURE
EPS = 1e-8
BIG = 1e9           # sim[diag] -= BIG before exp => exp -> 0

F32 = mybir.dt.float32
BF16 = mybir.dt.bfloat16
DT_MM = mybir.dt.float8e4

_build_cache = {}


def build(reps: int = 1, b: int = 512):
    """b: label boundary offset within each 1024-col slice (0 <= b < 1024)."""
    key = (reps, b)
    if key in _build_cache:
        return _build_cache[key]
    dt_mm = DT_MM
    ns = 2 if b > 0 else 1   # accum slots per (ic, jp)
    NW = NJ2 * ns            # accum slots per ic (row-chunk)

    nc = bacc.Bacc("TRN2", target_bir_lowering=False, debug=False)
    embT_d = nc.dram_tensor("embT", [D, N], dt_mm, kind="ExternalInput")
    labt_d = nc.dram_tensor("labt", [P, IC], F32, kind="ExternalInput")
    w_d = nc.dram_tensor("wpos", [P, IC * NW], F32, kind="ExternalInput")
    partial_d = nc.dram_tensor("partial", [1, 1], F32, kind="ExternalOutput")

    # [D, N] viewed as [p, dc, n] with d = dc*128 + p
    embT = embT_d.ap().rearrange("(dc p) n -> p dc n", p=P)

    with tile.TileContext(nc) as tc:
        with (
            tc.tile_pool(name="consts", bufs=1) as consts,
            tc.tile_pool(name="rhsp", bufs=3) as rhsp,
            tc.tile_pool(name="expp", bufs=2) as expp,
            tc.tile_pool(name="stats", bufs=1) as stats,
            tc.tile_pool(name="fin", bufs=1) as fin,
            tc.tile_pool(name="psum", bufs=3, space=bass.MemorySpace.PSUM) as psum,
            tc.tile_pool(name="fpsum", bufs=1, space=bass.MemorySpace.PSUM) as fpsum,
        ):
            # resident stationary operand: this core's 1024 embedding columns
            res = consts.tile([P, ND, ROWS], dt_mm)
            nc.sync.dma_start(out=res, in_=embT[:, :, 0:ROWS])
            # per-row label mask in [p, ic] layout
            labt = consts.tile([P, IC], F32)
            nc.sync.dma_start(out=labt, in_=labt_d.ap())
            # per-slot positive-membership weights (broadcast-tiled by host)
            wpos = consts.tile([P, IC * NW], F32)
            nc.sync.dma_start(out=wpos, in_=w_d.ap())
            # BIG * identity (subtracted on the diagonal block pre-exp)
            bigI = consts.tile([P, P], F32)
            nc.gpsimd.memset(bigI, 0.0)
            nc.gpsimd.affine_select(
                out=bigI,
                in_=bigI,
                compare_op=mybir.AluOpType.not_equal,
                fill=BIG,
                base=0,
                pattern=[[-1, P]],
                channel_multiplier=1,
            )
            ones = consts.tile([P, 1], F32)
            nc.vector.memset(ones, 1.0)
            epsb = consts.tile([P, 1], F32)
            nc.vector.memset(epsb, EPS)

            for rep in range(reps):
                alls = stats.tile([P, IC * NW], F32, tag="alls")

                for jp in range(NJ2):
                    rhs = rhsp.tile([P, ND, 2 * JS], dt_mm, tag="rhs")
                    nc.sync.dma_start(
                        out=rhs, in_=embT[:, :, jp * 2 * JS : (jp + 1) * 2 * JS]
                    )
                    for ic in range(IC):
                        # 4 DoubleRow matmuls, each streaming the full
                        # 1024-wide j range into a 2-bank PSUM tile (halves
                        # the ldweights count vs two 512-wide groups)
                        ps = psum.tile([P, 2, JS], F32, tag="ps")
                        for dc2 in range(ND // 2):
                            nc.tensor.matmul(
                                ps.rearrange("p s j -> p (s j)"),
                                res[:, 2 * dc2 : 2 * dc2 + 2, ic * P : (ic + 1) * P],
                                rhs[:, 2 * dc2 : 2 * dc2 + 2, :],
                                start=(dc2 == 0),
                                stop=(dc2 == ND // 2 - 1),
                                perf_mode=mybir.MatmulPerfMode.DoubleRow,
                            )
                        # diagonal block of this core sits at local columns
                        # [ic*128, ic*128+128): half ic//4, offset (ic*128)%512
                        if jp == 0:
                            off = (ic * P) % JS
                            nc.vector.tensor_sub(
                                ps[:, ic // 4, off : off + P],
                                ps[:, ic // 4, off : off + P],
                                bigI,
                            )
                        ext = expp.tile([P, 2 * JS], BF16, tag="ext")
                        flat = ps.rearrange("p s j -> p (s j)")
                        idx = (ic * NJ2 + jp) * ns
                        if ns == 2:
                            nc.scalar.activation(
                                out=ext[:, 0:b],
                                in_=flat[:, 0:b],
                                func=mybir.ActivationFunctionType.Exp,
                                scale=SCALE,
                                accum_out=alls[:, idx : idx + 1],
                            )
                            nc.scalar.activation(
                                out=ext[:, b : 2 * JS],
                                in_=flat[:, b : 2 * JS],
                                func=mybir.ActivationFunctionType.Exp,
                                scale=SCALE,
                                accum_out=alls[:, idx + 1 : idx + 2],
                            )
                        else:
                            nc.scalar.activation(
                                out=ext,
                                in_=flat,
                                func=mybir.ActivationFunctionType.Exp,
                                scale=SCALE,
                                accum_out=alls[:, idx : idx + 1],
                            )

                # ---- per-row loss and partial reduction ----
                asum = fin.tile([P, IC], F32, tag="asum")
                nc.vector.reduce_sum(
                    asum,
                    alls.rearrange("p (ic k) -> p ic k", k=NW),
                    axis=mybir.AxisListType.X,
                )
                wrk = fin.tile([P, IC * NW], F32, tag="wrk")
                nc.vector.scalar_tensor_tensor(
                    out=wrk,
                    in0=alls,
                    scalar=1.0,
                    in1=wpos,
                    op0=mybir.AluOpType.mult,
                    op1=mybir.AluOpType.mult,
                )
                psumr = fin.tile([P, IC], F32, tag="psumr")
                nc.vector.reduce_sum(
                    psumr,
                    wrk.rearrange("p (ic k) -> p ic k", k=NW),
                    axis=mybir.AxisListType.X,
                )
                lnall = fin.tile([P, IC], F32, tag="lnall")
                nc.scalar.activation(
                    out=lnall,
                    in_=asum,
                    func=mybir.ActivationFunctionType.Ln,
                    bias=epsb,
                )
                lnpos = fin.tile([P, IC], F32, tag="lnpos")
                nc.scalar.activation(
                    out=lnpos,
                    in_=psumr,
                    func=mybir.ActivationFunctionType.Ln,
                )
                contrib = fin.tile([P, IC], F32, tag="contrib")
                nc.vector.tensor_sub(contrib, lnall, lnpos)
                nc.vector.tensor_mul(contrib, contrib, labt)
                # partition reduction via ones-matmul (fp32, tiny)
                fps = fpsum.tile([1, IC], F32, tag="fps")
                nc.tensor.matmul(fps, ones, contrib, start=True, stop=True)
                stot = fin.tile([1, 1], F32, tag="stot")
                nc.vector.reduce_sum(stot, fps, axis=mybir.AxisListType.X)
                nc.sync.dma_start(out=partial_d.ap(), in_=stot)

    nc.compile()
    _build_cache[key] = nc
    return nc


def make_in_maps(embeddings: np.ndarray, labels: np.ndarray):
    """Returns (in_maps, b)."""
    emb = np.asarray(embeddings, dtype=np.float32)
    lab = np.asarray(labels).astype(np.int32)
    order = np.argsort(lab, kind="stable")  # zeros first, ones after
    emb_s = emb[order]
    lab_s = lab[order].astype(np.float32)
    n0 = int(np.sum(lab_s == 0))
    b = n0 % (2 * JS)

    embT = np.ascontiguousarray(emb_s.T)  # [D, N]
    np_dt = mybir.dt.np(DT_MM)
    ns = 2 if b > 0 else 1
    NW = NJ2 * ns
    in_maps = []
    for c in range(NCORES):
        embT_rot = np.roll(embT, -c * ROWS, axis=1)
        lab_rot = np.roll(lab_s, -c * ROWS)
        labt = np.ascontiguousarray(
            lab_s[c * ROWS : (c + 1) * ROWS].reshape(IC, P).T
        )
        # per-slot positive membership (label-pure by construction)
        w = np.zeros(NW, dtype=np.float32)
        for jp in range(NJ2):
            pieces = [(0, b), (b, 2 * JS)] if ns == 2 else [(0, 2 * JS)]
            for h, (lo, hi) in enumerate(pieces):
                seg = lab_rot[jp * 2 * JS + lo : jp * 2 * JS + hi]
                assert seg.min() == seg.max(), "slice piece not label-pure"
                w[jp * ns + h] = seg[0]
        w_tiled = np.broadcast_to(
            np.tile(w, IC)[None, :], (P, IC * NW)
        ).copy()
        in_maps.append(
            {
                "embT": np.ascontiguousarray(embT_rot).astype(np_dt),
                "labt": labt,
                "wpos": w_tiled,
            }
        )
    return in_maps, b


def kernel(embeddings: np.ndarray, labels: np.ndarray) -> np.ndarray:
    lab_f = np.asarray(labels).astype(np.float32)
    n_ref = float(lab_f.sum())
    if n_ref < 2:
        return np.float32(0.0)

    in_maps, b = make_in_maps(embeddings, labels)
    nc = build(reps=1, b=b)
    res = run_bass_kernel_spmd(nc, in_maps, core_ids=list(range(NCORES)))
    total = np.float32(0.0)
    for c in range(NCORES):
        total += res.results[c]["partial"][0, 0]
    loss = total / np.float32(max(n_ref, 1.0))
    return np.asarray(loss, dtype=np.float32)


# revision 4
# speedup vs baseline: 1.5280x; 1.5280x over previous
"""Contrastive loss (SupCon-style) on 8 Trainium2 NeuronCores.

Reference (N=8192, D=1024, T=0.1):
    sim = emb @ emb.T / T;  e = exp(sim)
    all_sum_i = sum_j e_ij - e_ii
    pos_sum_i = sum_j e_ij * lab_j - e_ii * lab_i
    loss = mean_{i: lab_i=1} [ log(all_sum_i + eps) - log(pos_sum_i) ]
    (0.0 if n_ref < 2)

Strategy: rows are pre-sorted by label on the host (the loss is invariant
under a joint row+column permutation of the similarity matrix), then split
across 8 cores (1024 rows each). Each core gets a column-ROTATED copy of the
sorted emb^T so its diagonal block lands at fixed local columns [0, 1024) --
identical SPMD instruction stream, row sums invariant under the rotation.

Because columns are label-sorted-then-rotated, every 1024-column slice is
label-constant except at one in-slice offset b = n0 % 1024 (identical for
all cores). Each exp activation is split at b, so every per-slice row-sum
emitted by the ScalarE accumulator is label-pure; pos_sum is then just a
per-core weighted reduction of the accumulator stats. This removes the
whole VectorE pos-sum pass over the [N/8, N] exp matrix that the v1 kernel
ran (it was ~40% of device time on HW).

Per core: sim tile [128 i x 512 j] = 4 accumulating fp8 DoubleRow matmuls;
diagonal positions get -BIG added pre-exp (exact self-exclusion); ScalarE
exp(scale*x) with accum_out emits the per-slice-piece row sums; final
log/mask/reduce on device down to one partial scalar per core; host sums 8
partials / n_ref.
"""

import numpy as np

import concourse.bass as bass
import concourse.tile as tile
import concourse.mybir as mybir
from concourse import bacc
from concourse.bass_utils import run_bass_kernel_spmd

N, D = 8192, 1024
NCORES = 8
ROWS = N // NCORES  # 1024 rows per core
P = 128             # partitions
JS = 512            # j-slice width (one PSUM bank of fp32)
NJ2 = N // (2 * JS)  # 8 j slices of 1024
ND = D // P         # 8 contraction chunks
IC = ROWS // P      # 8 row chunks per core
SCALE = 10.0        # 1 / TEMPERATURE
EPS = 1e-8
BIG = 1e9           # sim[diag] -= BIG before exp => exp -> 0

F32 = mybir.dt.float32
BF16 = mybir.dt.bfloat16
DT_MM = mybir.dt.float8e4

_build_cache = {}


def build(reps: int = 1, b: int = 512):
    """b: label boundary offset within each 1024-col slice (0 <= b < 1024)."""
    key = (reps, b)
    if key in _build_cache:
        return _build_cache[key]
    dt_mm = DT_MM
    ns = 2 if b > 0 else 1   # accum slots per (ic, jp)
    NW = NJ2 * ns            # accum slots per ic (row-chunk)

    nc = bacc.Bacc("TRN2", target_bir_lowering=False, debug=False)
    embT_d = nc.dram_tensor("embT", [D, N], dt_mm, kind="ExternalInput")
    labt_d = nc.dram_tensor("labt", [P, IC], F32, kind="ExternalInput")
    w_d = nc.dram_tensor("wpos", [P, IC * NW], F32, kind="ExternalInput")
    partial_d = nc.dram_tensor("partial", [1, 1], F32, kind="ExternalOutput")

    # [D, N] viewed as [p, dc, n] with d = dc*128 + p
    embT = embT_d.ap().rearrange("(dc p) n -> p dc n", p=P)

    with tile.TileContext(nc) as tc:
        with (
            tc.tile_pool(name="consts", bufs=1) as consts,
            tc.tile_pool(name="rhsp", bufs=3) as rhsp,
            tc.tile_pool(name="expp", bufs=2) as expp,
            tc.tile_pool(name="stats", bufs=1) as stats,
            tc.tile_pool(name="fin", bufs=1) as fin,
            tc.tile_pool(name="psum", bufs=3, space=bass.MemorySpace.PSUM) as psum,
            tc.tile_pool(name="fpsum", bufs=1, space=bass.MemorySpace.PSUM) as fpsum,
        ):
            # resident stationary operand: this core's 1024 embedding columns
            res = consts.tile([P, ND, ROWS], dt_mm)
            nc.sync.dma_start(out=res, in_=embT[:, :, 0:ROWS])
            # per-row label mask in [p, ic] layout
            labt = consts.tile([P, IC], F32)
            nc.sync.dma_start(out=labt, in_=labt_d.ap())
            # per-slot positive-membership weights (broadcast-tiled by host)
            wpos = consts.tile([P, IC * NW], F32)
            nc.sync.dma_start(out=wpos, in_=w_d.ap())
            # BIG * identity (subtracted on the diagonal block pre-exp)
            bigI = consts.tile([P, P], F32)
            nc.gpsimd.memset(bigI, 0.0)
            nc.gpsimd.affine_select(
                out=bigI,
                in_=bigI,
                compare_op=mybir.AluOpType.not_equal,
                fill=BIG,
                base=0,
                pattern=[[-1, P]],
                channel_multiplier=1,
            )
            ones = consts.tile([P, 1], F32)
            nc.vector.memset(ones, 1.0)
            epsb = consts.tile([P, 1], F32)
            nc.vector.memset(epsb, EPS)

            for rep in range(reps):
                alls = stats.tile([P, IC * NW], F32, tag="alls")

                for jp in range(NJ2):
                    rhs = rhsp.tile([P, ND, 2 * JS], dt_mm, tag="rhs")
                    nc.sync.dma_start(
                        out=rhs, in_=embT[:, :, jp * 2 * JS : (jp + 1) * 2 * JS]
                    )
                    for ic in range(IC):
                        # two matmul groups -> one 2-bank PSUM tile (the ISA
                        # caps a matmul's moving operand at 1024 elements)
                        ps = psum.tile([P, 2, JS], F32, tag="ps")
                        for s in range(2):
                            rhs_s = rhs[:, :, s * JS : (s + 1) * JS]
                            for dc2 in range(ND // 2):
                                nc.tensor.matmul(
                                    ps[:, s, :],
                                    res[:, 2 * dc2 : 2 * dc2 + 2, ic * P : (ic + 1) * P],
                                    rhs_s[:, 2 * dc2 : 2 * dc2 + 2, :],
                                    start=(dc2 == 0),
                                    stop=(dc2 == ND // 2 - 1),
                                    perf_mode=mybir.MatmulPerfMode.DoubleRow,
                                )
                        # diagonal block of this core sits at local columns
                        # [ic*128, ic*128+128): half ic//4, offset (ic*128)%512
                        if jp == 0:
                            off = (ic * P) % JS
                            nc.vector.tensor_sub(
                                ps[:, ic // 4, off : off + P],
                                ps[:, ic // 4, off : off + P],
                                bigI,
                            )
                        ext = expp.tile([P, 2 * JS], BF16, tag="ext")
                        flat = ps.rearrange("p s j -> p (s j)")
                        idx = (ic * NJ2 + jp) * ns
                        if ns == 2:
                            nc.scalar.activation(
                                out=ext[:, 0:b],
                                in_=flat[:, 0:b],
                                func=mybir.ActivationFunctionType.Exp,
                                scale=SCALE,
                                accum_out=alls[:, idx : idx + 1],
                            )
                            nc.scalar.activation(
                                out=ext[:, b : 2 * JS],
                                in_=flat[:, b : 2 * JS],
                                func=mybir.ActivationFunctionType.Exp,
                                scale=SCALE,
                                accum_out=alls[:, idx + 1 : idx + 2],
                            )
                        else:
                            nc.scalar.activation(
                                out=ext,
                                in_=flat,
                                func=mybir.ActivationFunctionType.Exp,
                                scale=SCALE,
                                accum_out=alls[:, idx : idx + 1],
                            )

                # ---- per-row loss and partial reduction ----
                asum = fin.tile([P, IC], F32, tag="asum")
                nc.vector.reduce_sum(
                    asum,
                    alls.rearrange("p (ic k) -> p ic k", k=NW),
                    axis=mybir.AxisListType.X,
                )
                wrk = fin.tile([P, IC * NW], F32, tag="wrk")
                nc.vector.scalar_tensor_tensor(
                    out=wrk,
                    in0=alls,
                    scalar=1.0,
                    in1=wpos,
                    op0=mybir.AluOpType.mult,
                    op1=mybir.AluOpType.mult,
                )
                psumr = fin.tile([P, IC], F32, tag="psumr")
                nc.vector.reduce_sum(
                    psumr,
                    wrk.rearrange("p (ic k) -> p ic k", k=NW),
                    axis=mybir.AxisListType.X,
                )
                lnall = fin.tile([P, IC], F32, tag="lnall")
                nc.scalar.activation(
                    out=lnall,
                    in_=asum,
                    func=mybir.ActivationFunctionType.Ln,
                    bias=epsb,
                )
                lnpos = fin.tile([P, IC], F32, tag="lnpos")
                nc.scalar.activation(
                    out=lnpos,
                    in_=psumr,
                    func=mybir.ActivationFunctionType.Ln,
                )
                contrib = fin.tile([P, IC], F32, tag="contrib")
                nc.vector.tensor_sub(contrib, lnall, lnpos)
                nc.vector.tensor_mul(contrib, contrib, labt)
                # partition reduction via ones-matmul (fp32, tiny)
                fps = fpsum.tile([1, IC], F32, tag="fps")
                nc.tensor.matmul(fps, ones, contrib, start=True, stop=True)
                stot = fin.tile([1, 1], F32, tag="stot")
                nc.vector.reduce_sum(stot, fps, axis=mybir.AxisListType.X)
                nc.sync.dma_start(out=partial_d.ap(), in_=stot)

    nc.compile()
    _build_cache[key] = nc
    return nc


def make_in_maps(embeddings: np.ndarray, labels: np.ndarray):
    """Returns (in_maps, b)."""
    emb = np.asarray(embeddings, dtype=np.float32)
    lab = np.asarray(labels).astype(np.int32)
    order = np.argsort(lab, kind="stable")  # zeros first, ones after
    emb_s = emb[order]
    lab_s = lab[order].astype(np.float32)
    n0 = int(np.sum(lab_s == 0))
    b = n0 % (2 * JS)

    embT = np.ascontiguousarray(emb_s.T)  # [D, N]
    np_dt = mybir.dt.np(DT_MM)
    ns = 2 if b > 0 else 1
    NW = NJ2 * ns
    in_maps = []
    for c in range(NCORES):
        embT_rot = np.roll(embT, -c * ROWS, axis=1)
        lab_rot = np.roll(lab_s, -c * ROWS)
        labt = np.ascontiguousarray(
            lab_s[c * ROWS : (c + 1) * ROWS].reshape(IC, P).T
        )
        # per-slot positive membership (label-pure by construction)
        w = np.zeros(NW, dtype=np.float32)
        for jp in range(NJ2):
            pieces = [(0, b), (b, 2 * JS)] if ns == 2 else [(0, 2 * JS)]
            for h, (lo, hi) in enumerate(pieces):
                seg = lab_rot[jp * 2 * JS + lo : jp * 2 * JS + hi]
                assert seg.min() == seg.max(), "slice piece not label-pure"
                w[jp * ns + h] = seg[0]
        w_tiled = np.broadcast_to(
            np.tile(w, IC)[None, :], (P, IC * NW)
        ).copy()
        in_maps.append(
            {
                "embT": np.ascontiguousarray(embT_rot).astype(np_dt),
                "labt": labt,
                "wpos": w_tiled,
            }
        )
    return in_maps, b


def kernel(embeddings: np.ndarray, labels: np.ndarray) -> np.ndarray:
    lab_f = np.asarray(labels).astype(np.float32)
    n_ref = float(lab_f.sum())
    if n_ref < 2:
        return np.float32(0.0)

    in_maps, b = make_in_maps(embeddings, labels)
    nc = build(reps=1, b=b)
    res = run_bass_kernel_spmd(nc, in_maps, core_ids=list(range(NCORES)))
    total = np.float32(0.0)
    for c in range(NCORES):
        total += res.results[c]["partial"][0, 0]
    loss = total / np.float32(max(n_ref, 1.0))
    return np.asarray(loss, dtype=np.float32)
